# revision 42
# baseline (speedup 1.0000x reference)
# Trainium2 Bass kernel for nn_EncoderBlock (dense transformer encoder block).
#
# Sharding: 8 cores, zero collectives. Core c owns batch b = c // 4 and query
# slice qs = (c % 4) * 512. Each core redundantly computes LN1/K/V for its
# whole batch (2048 tokens) and runs attention + FFN for its own 512 queries.
# The host rolls the token order per core so that the core's queries are
# tokens 0..511 of its view -- every core then runs the identical SPMD
# program with static offsets. The host also feeds x transposed ([D, S]),
# since fp32 has no DMA-transpose path on TRN2.
#
# Device dataflow (transposed activations: feature dim on partitions, tokens
# on the free dim; all matmul operands in bf16 = full PE speed):
#   LN1 (per 512-token chunk, pipelined): column sums of x and x*x via
#       ones-vector matmuls; the [1,512] stat rows are broadcast across
#       partitions with K=1 matmuls and the coefficient math (1/std etc.)
#       runs wide on [128,512] tiles instead of single-lane rows.
#   Q/K proj   : Wq/Wk stationary -> qT/kT [d, tokens]; qT is pre-scaled by
#                log2e*128/sqrt(dk) so softmax exp needs no multiply.
#   V proj     : ln1T tiles stationary, Wv moving -> v [tokens, d] (plain)
#   scores     : lhsT = kT head tile, rhs = qT head -> scoresT [kpos, q];
#                the two heads of a pair co-execute on PE row halves.
#   softmax    : per (pair, kpos-chunk), head 0's exp runs on ScalarE (true
#                Exp) while head 1's runs on VectorE as a bit-trick exp2
#                (i16 = s + magic, bitcast as bf16; ~3% relative error which
#                softmax normalization washes out; saturation to -32768 on a
#                masked -1e30 bias gives bf16 -0.0, i.e. exact masking).
#                Denominators come free from a ones column appended to V.
#   attn@v     : lhsT = [v_head | 1] [kpos, 65], rhs = expT -> [65, q]
#   Wo + resid, LN2 (same wide-coefficient scheme), FFN (relu fused into
#   PSUM eviction), resid, store.  All activation functions used (ln, exp,
#   relu, square, identity, copy) live in the single ACT table set
#   natural_log_exp_and_others, pinned at compile time so there are no
#   table reloads.

import numpy as np

D_MODEL = 512
H = 8
DK = 64
D_FF = 2048
B = 2
S = 2048
EPS = 1e-6

N_CORES = 8
CORES_PER_BATCH = 4
Q = 512          # queries per core
P = 128          # partitions
KD = D_MODEL // P      # 4 feature chunks
FJ = D_FF // P         # 16 ff chunks
TT = S // P            # 16 kpos chunks
TC = S // 512          # 4 token column chunks

LOG2E = 1.4426950408889634
EXP_A = float(LOG2E * 128.0 / np.sqrt(np.float64(DK)))   # folded into qT
EXP_MAGIC = 127.0 * 128.0 - 3.9
SC_SCALE = float(np.log(2.0) / 128.0)                    # ACT exp scale

_BUILT = None


def _emit(nc, tc, aps):
    import concourse.bass as bass
    from concourse import mybir

    f32 = mybir.dt.float32
    bf16 = mybir.dt.bfloat16
    i16 = mybir.dt.int16
    Act = mybir.ActivationFunctionType
    Op = mybir.AluOpType

    def r(ap):
        return ap

    xT, xq, mask, Wq, Wk, Wv, Wo, W1, W2, bq, bk, bv, bo, b1, b2, consts, outT = aps

    mm = nc.tensor.matmul

    p_small = tc.alloc_tile_pool(name="p_small", bufs=1, side="left")
    p_work = tc.alloc_tile_pool(name="p_work", bufs=4, side="left")
    p_wo = tc.alloc_tile_pool(name="p_wo", bufs=1, side="right")
    p_xq = tc.alloc_tile_pool(name="p_xq", bufs=1, side="right")
    p_wqkv = tc.alloc_tile_pool(name="p_wqkv", bufs=1, side="right")
    p_big = tc.alloc_tile_pool(name="p_big", bufs=1, side="right")

    # ---------------- constant / input loads ----------------
    ones_col = p_small.tile([P, 1], bf16)
    nc.vector.memset(ones_col, 1.0)
    ones_row = p_small.tile([1, P], bf16)
    nc.vector.memset(ones_row, 1.0)

    ln511_sb = p_small.tile([1, 1], f32)
    nc.vector.memset(ln511_sb, float(0.5 * np.log(np.float64(D_MODEL - 1))))
    consts_sb = p_small.tile([1, 4], f32)
    nc.sync.dma_start(out=consts_sb, in_=consts.rearrange("(o c) -> o c", o=1))
    a1_ap = consts_sb[0:1, 0:1]
    be1_ap = consts_sb[0:1, 1:2]
    a2_ap = consts_sb[0:1, 2:3]
    be2_ap = consts_sb[0:1, 3:4]

    mask_i = p_small.tile([P, TT], mybir.dt.int32)
    nc.sync.dma_start(out=mask_i, in_=mask.rearrange("(t p) -> p t", p=P))
    maskb = p_small.tile([P, TT], f32)
    nc.vector.tensor_copy(out=maskb, in_=mask_i)
    # mask 1 -> 0.0 ; mask 0 -> -1e30  (additive bias inside exp)
    nc.vector.tensor_scalar(
        out=maskb, in0=maskb, scalar1=1e30, scalar2=-1e30, op0=Op.mult, op1=Op.add
    )
    # VectorE fast-exp bias: i16 = s' + (maskb*log2e*128 + EXP_MAGIC) where
    # s' is the pre-scaled score; bitcast of that i16 as bf16 ~ exp(.).
    maskb2 = p_small.tile([P, TT], f32)
    nc.vector.tensor_scalar(
        out=maskb2, in0=maskb, scalar1=float(LOG2E * 128.0), scalar2=EXP_MAGIC,
        op0=Op.mult, op1=Op.add,
    )

    p_xT = tc.alloc_tile_pool(name="p_xT", bufs=1, side="right")
    xT_sb = p_xT.tile([P, KD, S], bf16)
    xT_r = xT.rearrange("(k p) t -> p k t", p=P)
    for t in range(TC):
        sl = slice(t * 512, (t + 1) * 512)
        for k in range(KD):
            nc.sync.dma_start(out=xT_sb[:, k, sl], in_=xT_r[:, k, sl])

    wq_sb = p_wqkv.tile([P, KD, D_MODEL], bf16)
    wk_sb = p_wqkv.tile([P, KD, D_MODEL], bf16)
    wv_sb = p_wqkv.tile([P, KD, D_MODEL], bf16)
    wo_sb = p_wo.tile([P, KD, D_MODEL], bf16)
    for w_sb, w in ((wq_sb, Wq), (wk_sb, Wk), (wv_sb, Wv), (wo_sb, Wo)):
        nc.sync.dma_start(out=w_sb, in_=w.rearrange("(k p) o -> p k o", p=P))
    bq_sb = p_wqkv.tile([P, KD], f32)
    bk_sb = p_wqkv.tile([P, KD], f32)
    bo_sb = p_wo.tile([P, KD], f32)
    nc.sync.dma_start(out=bq_sb, in_=bq.rearrange("(j p) -> p j", p=P))
    nc.sync.dma_start(out=bk_sb, in_=bk.rearrange("(j p) -> p j", p=P))
    nc.sync.dma_start(out=bo_sb, in_=bo.rearrange("(j p) -> p j", p=P))
    bv_f = p_wqkv.tile([1, D_MODEL], f32)
    nc.sync.dma_start(out=bv_f, in_=bv.rearrange("(o d) -> o d", o=1))
    bv_row = p_wqkv.tile([1, D_MODEL], bf16)
    nc.scalar.copy(out=bv_row, in_=bv_f)
    xq_sb = p_xq.tile([P, KD, Q], f32)
    nc.sync.dma_start(out=xq_sb, in_=xq.rearrange("(k p) t -> p k t", p=P))

    # ---------------- LN1 + QKV, pipelined per 512-token chunk ----------
    ps_st = tc.alloc_tile_pool(name="ps_st", bufs=1, space="PSUM")
    ps_b = tc.alloc_tile_pool(name="ps_b", bufs=2, space="PSUM")
    ps_mm = tc.alloc_tile_pool(name="ps_mm", bufs=2, space="PSUM")

    ln1 = p_big.tile([P, KD, S], bf16, tag="big")
    # pool + tiles for W1 sit below p_qkv on the left stack (so p_qkv can be
    # released first); the actual DMA is issued at the attention section.
    p_w1 = tc.alloc_tile_pool(name="p_w1", bufs=1, side="left")
    w1_sb = p_w1.tile([P, KD, D_FF], bf16)
    b1_sb = p_w1.tile([P, FJ], f32)
    p_qkv = tc.alloc_tile_pool(name="p_qkv", bufs=1, side="left")
    qT = p_qkv.tile([P, KD, Q], bf16)
    kT = p_qkv.tile([P, KD, S], bf16)
    v_sb = p_qkv.tile([P, TT, H, DK + 1], bf16)
    nc.gpsimd.memset(v_sb, 1.0)

    LN511 = float(0.5 * np.log(np.float64(D_MODEL - 1)))

    def ln_chunk(x_ap, xsq_ap, width, st_pool, b_pool, alpha_ap, beta_ap):
        # stats: column sums of x and x^2 over all KD partition chunks
        s1_ps = st_pool.tile([1, width], f32, tag="s1")
        s2_ps = st_pool.tile([1, width], f32, tag="s2")
        for k in range(KD):
            mm(s1_ps, r(ones_col), r(x_ap[:, k, :]), start=(k == 0),
               stop=(k == KD - 1))
        for k in range(KD):
            mm(s2_ps, r(ones_col), r(xsq_ap[:, k, :]), start=(k == 0),
               stop=(k == KD - 1))
        s1_row = p_work.tile([1, width], f32, tag="s1row", bufs=2)
        s2_row = p_work.tile([1, width], f32, tag="s2row", bufs=2)
        nc.scalar.copy(out=s1_row, in_=s1_ps)
        nc.scalar.copy(out=s2_row, in_=s2_ps)
        # single-lane f32 coefficient math on [1, width] rows (f32 keeps the
        # s2 - s1^2/n cancellation exact; a row op is only `width` cycles):
        # u = (n-1) var ; 1/std = exp(-.5 ln u + .5 ln(n-1)) ; a = alpha/std
        # c = beta - (s1/n) a
        n_tok = float(D_MODEL)
        t0 = p_work.tile([1, width], f32, tag="lnt0", bufs=2)
        nc.vector.scalar_tensor_tensor(
            out=t0, in0=s1_row, scalar=1.0 / n_tok, in1=s1_row,
            op0=Op.mult, op1=Op.mult,
        )
        u = p_work.tile([1, width], f32, tag="lnu", bufs=2)
        nc.vector.tensor_tensor(out=u, in0=s2_row, in1=t0, op=Op.subtract)
        l = p_work.tile([1, width], f32, tag="lnl", bufs=2)
        nc.scalar.activation(out=l, in_=u, func=Act.Ln)
        a0 = p_work.tile([1, width], f32, tag="lna0", bufs=2)
        nc.scalar.activation(out=a0, in_=l, func=Act.Exp, scale=-0.5, bias=ln511_sb[0:1, 0:1])
        a_row = p_work.tile([1, width], bf16, tag="lnar", bufs=2)
        nc.scalar.mul(out=a_row, in_=a0, mul=alpha_ap)
        cm = p_work.tile([1, width], f32, tag="lncm", bufs=2)
        nc.vector.scalar_tensor_tensor(
            out=cm, in0=s1_row, scalar=-1.0 / n_tok, in1=a_row,
            op0=Op.mult, op1=Op.mult,
        )
        c_row = p_work.tile([1, width], bf16, tag="lncr", bufs=2)
        nc.scalar.add(out=c_row, in_=cm, add=beta_ap)
        # broadcast a/c across partitions (K=1 matmuls), then evict to bf16
        # SBUF (ScalarE) so the wide LN apply runs in the DVE 2x 16-bit mode.
        a_ps = b_pool.tile([P, width], f32, tag="ab")
        c_ps = b_pool.tile([P, width], f32, tag="cb")
        mm(a_ps, r(ones_row), r(a_row), start=True, stop=True)
        mm(c_ps, r(ones_row), r(c_row), start=True, stop=True)
        a_sb = p_work.tile([P, width], bf16, tag="lnasb", bufs=2)
        c_sb = p_work.tile([P, width], bf16, tag="lncsb", bufs=2)
        nc.scalar.copy(out=a_sb, in_=a_ps)
        nc.scalar.copy(out=c_sb, in_=c_ps)
        return a_sb, c_sb

    for t in range(TC):
        sl = slice(t * 512, (t + 1) * 512)
        # x^2 for this chunk (GpSimd, SBUF->SBUF)
        xsq_t = p_work.tile([P, KD, 512], bf16, tag="xsq", bufs=2)
        for k in range(KD):
            nc.vector.tensor_tensor(
                out=xsq_t[:, k, :], in0=xT_sb[:, k, sl], in1=xT_sb[:, k, sl],
                op=Op.mult,
            )
        a_sb, c_sb = ln_chunk(
            xT_sb[:, :, sl], xsq_t, 512, ps_st, ps_b, a1_ap, be1_ap
        )
        # apply: ln1 = x * a + c  (all-bf16 SBUF operands -> DVE 2x mode)
        for k in range(KD):
            nc.vector.tensor_tensor(
                out=ln1[:, k, sl], in0=xT_sb[:, k, sl], in1=a_sb, op=Op.mult
            )
            nc.vector.tensor_tensor(
                out=ln1[:, k, sl], in0=ln1[:, k, sl], in1=c_sb, op=Op.add
            )
        if t == 0:
            # Q projection only needs the first chunk; qT pre-scaled by EXP_A
            for j in range(KD):
                q_ps = ps_mm.tile([P, 512], f32, tag="mm")
                for k in range(KD):
                    mm(q_ps, r(wq_sb[:, k, j * P:(j + 1) * P]), r(ln1[:, k, 0:Q]),
                       start=(k == 0), stop=(k == KD - 1))
                nc.vector.tensor_scalar(
                    out=qT[:, j, :], in0=q_ps, scalar1=bq_sb[:, j:j + 1],
                    scalar2=EXP_A, op0=Op.add, op1=Op.mult,
                )
        # K projection for this chunk (bias in the DVE eviction)
        for j in range(KD):
            k_ps = ps_mm.tile([P, 512], f32, tag="mm")
            for k in range(KD):
                mm(k_ps, r(wk_sb[:, k, j * P:(j + 1) * P]), r(ln1[:, k, sl]),
                   start=(k == 0), stop=(k == KD - 1))
            nc.vector.tensor_scalar_add(
                out=kT[:, j, sl], in0=k_ps, scalar1=bk_sb[:, j:j + 1]
            )
        # V projection for this chunk (4 kpos tiles of 128 tokens); bv is
        # added by a K=1 ones matmul so the eviction is a plain ScalarE copy
        for tt in range(4 * t, 4 * t + 4):
            v_ps = ps_mm.tile([P, 512], f32, tag="mm")
            for k in range(KD):
                mm(v_ps, r(ln1[:, k, tt * P:(tt + 1) * P]), r(wv_sb[:, k, :]),
                   start=(k == 0), stop=False)
            mm(v_ps, r(ones_row), r(bv_row), start=False, stop=True)
            nc.scalar.copy(
                out=v_sb[:, tt, :, 0:DK],
                in_=v_ps.rearrange("p (h d) -> p h d", h=H),
            )

    p_xT.release()
    p_big.release()
    p_wqkv.release()

    # ---------------- attention ----------------
    ps_mm.release()
    ps_b.release()
    ps_st.release()
    p_attn = tc.alloc_tile_pool(name="p_attn", bufs=1, side="right")
    attn_sb = p_attn.tile([P, KD, Q], bf16)
    ps_sc = tc.alloc_tile_pool(name="ps_sc", bufs=3, space="PSUM")
    ps_ov = tc.alloc_tile_pool(name="ps_ov", bufs=1, space="PSUM")

    # W1 prefetch overlaps attention
    nc.sync.dma_start(out=w1_sb, in_=W1.rearrange("(k p) o -> p k o", p=P))
    nc.sync.dma_start(out=b1_sb, in_=b1.rearrange("(j p) -> p j", p=P))

    # Head pairs: 2 score MMs co-execute on PE row halves into one [P,1024]
    # PSUM tile; exp units alternate between ScalarE (true Exp) and VectorE
    # (bit-trick exp2) so neither engine paces the PE.  The attn@v matmuls
    # are emitted SKEW units behind the scores: the PE queue is in-order, so
    # without the skew a unit's attn@v (waiting on its exp) would block the
    # next unit's already-runnable score matmuls behind it.
    EXP_ACT_UNITS = 33          # of 64 (pair, tt) units on ScalarE
    SKEW = 2
    exp_acc = 0
    ov_tiles = {}
    pending = []

    def emit_attnv(u):
        pj, tt, expT = u
        if tt == 0:
            ov0_t = ps_ov.tile([DK + 1, 512], f32, tag="ov0", name=f"ov0_{pj}")
            ov1_t = ps_ov.tile([DK + 1, 512], f32, tag="ov1", name=f"ov1_{pj}")
            ov_tiles[pj] = (ov0_t, ov1_t)
        ov0, ov1 = ov_tiles[pj]
        mm(ov0, r(v_sb[:, tt, 2 * pj, :]), r(expT[:, 0:512]),
           start=(tt == 0), stop=(tt == TT - 1))
        mm(ov1, r(v_sb[:, tt, 2 * pj + 1, :]), r(expT[:, 512:1024]),
           start=(tt == 0), stop=(tt == TT - 1))
        if tt == TT - 1:
            # epilogue: evict, 1/denominator, broadcast, normalize
            ovs0 = p_attn.tile([DK + 1, 512], f32, tag=f"ovs{pj}_0")
            nc.scalar.copy(out=ovs0, in_=ov0)
            ovs1 = p_attn.tile([DK + 1, 512], f32, tag=f"ovs{pj}_1")
            nc.vector.tensor_copy(out=ovs1, in_=ov1)
            for po, ovs in ((0, ovs0), (DK, ovs1)):
                recip = p_attn.tile([1, 512], f32, tag=f"recip{pj}_{po}")
                nc.scalar.activation(out=recip, in_=ovs[DK:DK + 1, :],
                                     func=Act.Ln)
                nc.scalar.activation(out=recip, in_=recip, func=Act.Exp,
                                     scale=-1.0)
                rb = p_work.tile([DK, 512], f32, tag="rb")
                nc.gpsimd.partition_broadcast(out_ap=rb, in_ap=recip)
                nc.vector.tensor_tensor(
                    out=attn_sb[po:po + DK, pj, :], in0=ovs[0:DK, :], in1=rb,
                    op=Op.mult,
                )

    for pj in range(H // 2):
        for tt in range(TT):
            sc_ps = ps_sc.tile([P, 1024], f32, tag="sc")
            mm(sc_ps[:, 0:512],
               r(kT[0:DK, pj, tt * P:(tt + 1) * P]),
               r(qT[0:DK, pj, :]),
               start=True, stop=True, tile_position=(0, 0))
            mm(sc_ps[:, 512:1024],
               r(kT[DK:P, pj, tt * P:(tt + 1) * P]),
               r(qT[DK:P, pj, :]),
               start=True, stop=True, tile_position=(64, 0))
            # whole [128,1024] exp per unit, alternating engines: large
            # instructions amortize fixed cost vs per-half splits, and
            # keeping combined exp throughput under PE's unit time keeps
            # the PE dense (HAM stays at K=8/8).
            expT = p_work.tile([P, 1024], bf16, tag="expT")
            exp_acc += EXP_ACT_UNITS
            if exp_acc >= 64:
                exp_acc -= 64
                nc.scalar.activation(
                    out=expT, in_=sc_ps, func=Act.Exp,
                    bias=maskb[:, tt:tt + 1], scale=SC_SCALE,
                )
            else:
                nc.vector.tensor_scalar_add(
                    out=expT.bitcast(i16), in0=sc_ps,
                    scalar1=maskb2[:, tt:tt + 1],
                )
            pending.append((pj, tt, expT))
            if len(pending) > SKEW:
                emit_attnv(pending.pop(0))
        # flush at the pair boundary: deferring a pair's last attn@v past the
        # next pair's scores entangles the epilogue with the ov-bank reuse
        # (observed to hang the device at SKEW=2)
        while pending:
            emit_attnv(pending.pop(0))

    ps_ov.release()
    ps_sc.release()
    p_qkv.release()
    ps_mm2 = tc.alloc_tile_pool(name="ps_mm2", bufs=3, space="PSUM")
    ps_st2 = tc.alloc_tile_pool(name="ps_st2", bufs=1, space="PSUM")
    ps_b2 = tc.alloc_tile_pool(name="ps_b2", bufs=1, space="PSUM")

    # W2 load overlaps Wo / LN2
    p_w2 = tc.alloc_tile_pool(name="p_w2", bufs=1, side="left")
    w2_sb = p_w2.tile([P, FJ, D_MODEL], bf16)
    nc.sync.dma_start(out=w2_sb, in_=W2.rearrange("(k p) o -> p k o", p=P))
    b2_sb = p_w2.tile([P, KD], f32)
    nc.sync.dma_start(out=b2_sb, in_=b2.rearrange("(j p) -> p j", p=P))

    # ---------------- Wo + residual -> x2, LN2 stats per j ----------------
    p_x2 = tc.alloc_tile_pool(name="p_x2", bufs=1, side="left")
    x2_sb = p_x2.tile([P, KD, Q], f32)
    x2b = p_x2.tile([P, KD, Q], bf16)
    x2sq = p_x2.tile([P, KD, Q], bf16)
    s1q_ps = ps_st2.tile([1, Q], f32, tag="s1")
    s2q_ps = ps_st2.tile([1, Q], f32, tag="s2")
    for j in range(KD):
        o_ps = ps_mm2.tile([P, 512], f32, tag="mm")
        for k in range(KD):
            mm(o_ps, r(wo_sb[:, k, j * P:(j + 1) * P]), r(attn_sb[:, k, :]),
               start=(k == 0), stop=(k == KD - 1))
        nc.vector.scalar_tensor_tensor(
            out=x2_sb[:, j, :], in0=o_ps, scalar=bo_sb[:, j:j + 1],
            in1=xq_sb[:, j, :], op0=Op.add, op1=Op.add,
        )
        nc.vector.tensor_copy(out=x2b[:, j, :], in_=x2_sb[:, j, :])
        nc.vector.tensor_tensor(
            out=x2sq[:, j, :], in0=x2b[:, j, :], in1=x2b[:, j, :], op=Op.mult
        )
        mm(s1q_ps, r(ones_col), r(x2b[:, j, :]), start=(j == 0),
           stop=(j == KD - 1))
    for j in range(KD):
        mm(s2q_ps, r(ones_col), r(x2sq[:, j, :]), start=(j == 0),
           stop=(j == KD - 1))

    p_attn.release()
    p_xq.release()
    p_wo.release()

    # ---------------- LN2 (same single-lane coeff scheme) ----------------
    s1q_row = p_work.tile([1, Q], f32, tag="s1row", bufs=2)
    s2q_row = p_work.tile([1, Q], f32, tag="s2row", bufs=2)
    nc.scalar.copy(out=s1q_row, in_=s1q_ps)
    nc.scalar.copy(out=s2q_row, in_=s2q_ps)
    n_tok = float(D_MODEL)
    t0 = p_work.tile([1, Q], f32, tag="lnt0", bufs=2)
    nc.vector.scalar_tensor_tensor(
        out=t0, in0=s1q_row, scalar=1.0 / n_tok, in1=s1q_row,
        op0=Op.mult, op1=Op.mult,
    )
    u = p_work.tile([1, Q], f32, tag="lnu", bufs=2)
    nc.vector.tensor_tensor(out=u, in0=s2q_row, in1=t0, op=Op.subtract)
    l = p_work.tile([1, Q], f32, tag="lnl", bufs=2)
    nc.scalar.activation(out=l, in_=u, func=Act.Ln)
    a0 = p_work.tile([1, Q], f32, tag="lna0", bufs=2)
    nc.scalar.activation(out=a0, in_=l, func=Act.Exp, scale=-0.5, bias=ln511_sb[0:1, 0:1])
    a2_row = p_work.tile([1, Q], bf16, tag="lnar", bufs=2)
    nc.scalar.mul(out=a2_row, in_=a0, mul=a2_ap)
    cm = p_work.tile([1, Q], f32, tag="lncm", bufs=2)
    nc.vector.scalar_tensor_tensor(
        out=cm, in0=s1q_row, scalar=-1.0 / n_tok, in1=a2_row,
        op0=Op.mult, op1=Op.mult,
    )
    c2_row = p_work.tile([1, Q], bf16, tag="lncr", bufs=2)
    nc.scalar.add(out=c2_row, in_=cm, add=be2_ap)
    a2_ps = ps_b2.tile([P, Q], f32, tag="ab")
    c2_ps = ps_b2.tile([P, Q], f32, tag="cb")
    mm(a2_ps, r(ones_row), r(a2_row), start=True, stop=True)
    mm(c2_ps, r(ones_row), r(c2_row), start=True, stop=True)
    a2_sb = p_work.tile([P, Q], bf16, tag="lnasb", bufs=2)
    c2_sb = p_work.tile([P, Q], bf16, tag="lncsb", bufs=2)
    nc.scalar.copy(out=a2_sb, in_=a2_ps)
    nc.scalar.copy(out=c2_sb, in_=c2_ps)

    # PE keep-warm: the LN2 coefficient chain leaves the PE idle for several
    # us, long enough for HAM to re-throttle the clock right before the FFN.
    # A few dependency-free dummy matmuls (no reader) keep its activity
    # window busy through the bubble.
    warm_ps = ps_mm2.tile([P, 512], f32, tag="warm", bufs=1)
    for _ in range(8):
        mm(warm_ps[0:1, :], r(ones_col), r(w1_sb[:, 0, 0:512]),
           start=True, stop=True)

    ln2 = p_x2.tile([P, KD, Q], bf16)
    for k in range(KD):
        nc.vector.tensor_tensor(out=ln2[:, k, :], in0=x2b[:, k, :], in1=a2_sb,
                                op=Op.mult)
        nc.vector.tensor_tensor(out=ln2[:, k, :], in0=ln2[:, k, :], in1=c2_sb,
                                op=Op.add)

    # ---------------- FFN ----------------
    p_h = tc.alloc_tile_pool(name="p_h", bufs=1, side="left")
    hT = p_h.tile([P, FJ, Q], bf16)
    for j in range(FJ):
        h_ps = ps_mm2.tile([P, 512], f32, tag="mm")
        for k in range(KD):
            mm(h_ps, r(w1_sb[:, k, j * P:(j + 1) * P]), r(ln2[:, k, :]),
               start=(k == 0), stop=(k == KD - 1))
        nc.scalar.activation(
            out=hT[:, j, :], in_=h_ps, func=Act.Relu, bias=b1_sb[:, j:j + 1],
            scale=1.0,
        )

    for j in range(KD):
        f_ps = ps_mm2.tile([P, 512], f32, tag="mm")
        for k in range(FJ):
            mm(f_ps, r(w2_sb[:, k, j * P:(j + 1) * P]), r(hT[:, k, :]),
               start=(k == 0), stop=(k == FJ - 1))
        o_sb = p_work.tile([P, 512], f32, tag="osb")
        nc.vector.scalar_tensor_tensor(
            out=o_sb, in0=f_ps, scalar=b2_sb[:, j:j + 1], in1=x2_sb[:, j, :],
            op0=Op.add, op1=Op.add,
        )
        nc.sync.dma_start(
            out=outT.rearrange("(j p) q -> p j q", p=P)[:, j, :], in_=o_sb
        )

    for pool in (p_h, p_x2, p_w2, p_w1, ps_b2, ps_st2, p_work, p_small, ps_mm2):
        pool.release()


def _build():
    global _BUILT
    if _BUILT is not None:
        return _BUILT
    import concourse.bass as bass
    import concourse.tile as tile
    from concourse import bacc, mybir
    from concourse._compat import axon_active

    f32 = mybir.dt.float32
    bf16 = mybir.dt.bfloat16
    i32 = mybir.dt.int32
    nc = bacc.Bacc(
        "TRN2",
        target_bir_lowering=False,
        debug=False,
        enable_asserts=False,
        num_devices=N_CORES,
    )
    aps = [
        nc.dram_tensor("xT", [D_MODEL, S], bf16, kind="ExternalInput").ap(),
        nc.dram_tensor("xq", [D_MODEL, Q], f32, kind="ExternalInput").ap(),
        nc.dram_tensor("mask", [S], i32, kind="ExternalInput").ap(),
        nc.dram_tensor("Wq", [D_MODEL, D_MODEL], bf16, kind="ExternalInput").ap(),
        nc.dram_tensor("Wk", [D_MODEL, D_MODEL], bf16, kind="ExternalInput").ap(),
        nc.dram_tensor("Wv", [D_MODEL, D_MODEL], bf16, kind="ExternalInput").ap(),
        nc.dram_tensor("Wo", [D_MODEL, D_MODEL], bf16, kind="ExternalInput").ap(),
        nc.dram_tensor("W1", [D_MODEL, D_FF], bf16, kind="ExternalInput").ap(),
        nc.dram_tensor("W2", [D_FF, D_MODEL], bf16, kind="ExternalInput").ap(),
        nc.dram_tensor("bq", [D_MODEL], f32, kind="ExternalInput").ap(),
        nc.dram_tensor("bk", [D_MODEL], f32, kind="ExternalInput").ap(),
        nc.dram_tensor("bv", [D_MODEL], f32, kind="ExternalInput").ap(),
        nc.dram_tensor("bo", [D_MODEL], f32, kind="ExternalInput").ap(),
        nc.dram_tensor("b1", [D_FF], f32, kind="ExternalInput").ap(),
        nc.dram_tensor("b2", [D_MODEL], f32, kind="ExternalInput").ap(),
        nc.dram_tensor("consts", [4], f32, kind="ExternalInput").ap(),
        nc.dram_tensor("outT", [D_MODEL, Q], f32, kind="ExternalOutput").ap(),
    ]
    with tile.TileContext(nc) as tc:
        _emit(nc, tc, aps)
    # Pin the single ACT table set that contains every function used here
    # (ln, exp, relu, square, identity, copy) so the per-instruction chooser
    # cannot ping-pong between sets (each reload costs ~2.7us serialized on
    # ScalarE).  Indices stay aligned with act_info.json.
    import concourse.bacc as bacc_mod

    orig_tables = bacc_mod.get_activation_tables

    def _one_set_tables(arch):
        tabs = orig_tables(arch)
        keep = "natural_log_exp_and_others"
        return {k: (v if k == keep else set()) for k, v in tabs.items()}

    bacc_mod.get_activation_tables = _one_set_tables
    try:
        nc.compile()
    finally:
        bacc_mod.get_activation_tables = orig_tables
    _BUILT = nc
    return nc


def make_in_maps(inputs):
    import ml_dtypes

    bf16 = ml_dtypes.bfloat16
    x = np.asarray(inputs["x"], np.float32)
    src_mask = np.asarray(inputs["src_mask"], np.int32)
    shared = {
        "Wq": np.ascontiguousarray(np.asarray(inputs["Wq"], np.float32).astype(bf16)),
        "Wk": np.ascontiguousarray(np.asarray(inputs["Wk"], np.float32).astype(bf16)),
        "Wv": np.ascontiguousarray(np.asarray(inputs["Wv"], np.float32).astype(bf16)),
        "Wo": np.ascontiguousarray(np.asarray(inputs["Wo"], np.float32).astype(bf16)),
        "W1": np.ascontiguousarray(np.asarray(inputs["W1"], np.float32).astype(bf16)),
        "W2": np.ascontiguousarray(np.asarray(inputs["W2"], np.float32).astype(bf16)),
        "bq": np.ascontiguousarray(np.asarray(inputs["bq"], np.float32)),
        "bk": np.ascontiguousarray(np.asarray(inputs["bk"], np.float32)),
        "bv": np.ascontiguousarray(np.asarray(inputs["bv"], np.float32)),
        "bo": np.ascontiguousarray(np.asarray(inputs["bo"], np.float32)),
        "b1": np.ascontiguousarray(np.asarray(inputs["b1"], np.float32)),
        "b2": np.ascontiguousarray(np.asarray(inputs["b2"], np.float32)),
        "consts": np.ascontiguousarray(
            np.array(
                [
                    np.asarray(inputs["alpha1"]).reshape(-1)[0],
                    np.asarray(inputs["beta1"]).reshape(-1)[0],
                    np.asarray(inputs["alpha2"]).reshape(-1)[0],
                    np.asarray(inputs["beta2"]).reshape(-1)[0],
                ],
                np.float32,
            )
        ),
    }
    in_maps = []
    for c in range(N_CORES):
        b = c // CORES_PER_BATCH
        qs = (c % CORES_PER_BATCH) * Q
        x_rot = np.concatenate([x[b, qs:, :], x[b, :qs, :]], axis=0)
        m_b = src_mask[b, 0, 0, :]
        m_rot = np.concatenate([m_b[qs:], m_b[:qs]], axis=0)
        in_map = dict(shared)
        in_map["xT"] = np.ascontiguousarray(x_rot.T.astype(bf16))
        in_map["xq"] = np.ascontiguousarray(x_rot[0:Q, :].T)
        in_map["mask"] = np.ascontiguousarray(m_rot)
        in_maps.append(in_map)
    return in_maps


def assemble_output(results):
    out = np.empty((B, S, D_MODEL), np.float32)
    for c in range(N_CORES):
        b = c // CORES_PER_BATCH
        qs = (c % CORES_PER_BATCH) * Q
        out[b, qs:qs + Q, :] = results[c]["outT"].T
    return out


def kernel(**inputs):
    from concourse.bass_utils import run_bass_kernel_spmd

    nc = _build()
    in_maps = make_in_maps(inputs)
    res = run_bass_kernel_spmd(nc, in_maps, core_ids=list(range(N_CORES)))
    return assemble_output(res.results)


# revision 43
# speedup vs baseline: 1.0126x; 1.0126x over previous
# Trainium2 Bass kernel for nn_EncoderBlock (dense transformer encoder block).
#
# Sharding: 8 cores, zero collectives. Core c owns batch b = c // 4 and query
# slice qs = (c % 4) * 512. Each core redundantly computes LN1/K/V for its
# whole batch (2048 tokens) and runs attention + FFN for its own 512 queries.
# The host rolls the token order per core so that the core's queries are
# tokens 0..511 of its view -- every core then runs the identical SPMD
# program with static offsets. The host also feeds x transposed ([D, S]),
# since fp32 has no DMA-transpose path on TRN2.
#
# Device dataflow (transposed activations: feature dim on partitions, tokens
# on the free dim; all matmul operands in bf16 = full PE speed):
#   LN1 (per 512-token chunk, pipelined): column sums of x and x*x via
#       ones-vector matmuls; the [1,512] stat rows are broadcast across
#       partitions with K=1 matmuls and the coefficient math (1/std etc.)
#       runs wide on [128,512] tiles instead of single-lane rows.
#   Q/K proj   : Wq/Wk stationary -> qT/kT [d, tokens]; qT is pre-scaled by
#                log2e*128/sqrt(dk) so softmax exp needs no multiply.
#   V proj     : ln1T tiles stationary, Wv moving -> v [tokens, d] (plain)
#   scores     : lhsT = kT head tile, rhs = qT head -> scoresT [kpos, q];
#                the two heads of a pair co-execute on PE row halves.
#   softmax    : per (pair, kpos-chunk), head 0's exp runs on ScalarE (true
#                Exp) while head 1's runs on VectorE as a bit-trick exp2
#                (i16 = s + magic, bitcast as bf16; ~3% relative error which
#                softmax normalization washes out; saturation to -32768 on a
#                masked -1e30 bias gives bf16 -0.0, i.e. exact masking).
#                Denominators come free from a ones column appended to V.
#   attn@v     : lhsT = [v_head | 1] [kpos, 65], rhs = expT -> [65, q]
#   Wo + resid, LN2 (same wide-coefficient scheme), FFN (relu fused into
#   PSUM eviction), resid, store.  All activation functions used (ln, exp,
#   relu, square, identity, copy) live in the single ACT table set
#   natural_log_exp_and_others, pinned at compile time so there are no
#   table reloads.

import numpy as np

D_MODEL = 512
H = 8
DK = 64
D_FF = 2048
B = 2
S = 2048
EPS = 1e-6

N_CORES = 8
CORES_PER_BATCH = 4
Q = 512          # queries per core
P = 128          # partitions
KD = D_MODEL // P      # 4 feature chunks
FJ = D_FF // P         # 16 ff chunks
TT = S // P            # 16 kpos chunks
TC = S // 512          # 4 token column chunks

LOG2E = 1.4426950408889634
EXP_A = float(LOG2E * 128.0 / np.sqrt(np.float64(DK)))   # folded into qT
EXP_MAGIC = 127.0 * 128.0 - 3.9
SC_SCALE = float(np.log(2.0) / 128.0)                    # ACT exp scale

_BUILT = None


def _emit(nc, tc, aps):
    import concourse.bass as bass
    from concourse import mybir

    f32 = mybir.dt.float32
    bf16 = mybir.dt.bfloat16
    i16 = mybir.dt.int16
    Act = mybir.ActivationFunctionType
    Op = mybir.AluOpType

    def r(ap):
        return ap

    xT, xq, mask, Wq, Wk, Wv, Wo, W1, W2, bq, bk, bv, bo, b1, b2, consts, outT = aps

    mm = nc.tensor.matmul

    p_small = tc.alloc_tile_pool(name="p_small", bufs=1, side="left")
    p_work = tc.alloc_tile_pool(name="p_work", bufs=4, side="left")
    p_wo = tc.alloc_tile_pool(name="p_wo", bufs=1, side="right")
    p_xq = tc.alloc_tile_pool(name="p_xq", bufs=1, side="right")
    p_wqkv = tc.alloc_tile_pool(name="p_wqkv", bufs=1, side="right")
    p_big = tc.alloc_tile_pool(name="p_big", bufs=1, side="right")

    # ---------------- constant / input loads ----------------
    ones_col = p_small.tile([P, 1], bf16)
    nc.vector.memset(ones_col, 1.0)
    ones_row = p_small.tile([1, P], bf16)
    nc.vector.memset(ones_row, 1.0)

    ln511_sb = p_small.tile([1, 1], f32)
    nc.vector.memset(ln511_sb, float(0.5 * np.log(np.float64(D_MODEL - 1))))
    consts_sb = p_small.tile([1, 4], f32)
    nc.sync.dma_start(out=consts_sb, in_=consts.rearrange("(o c) -> o c", o=1))
    a1_ap = consts_sb[0:1, 0:1]
    be1_ap = consts_sb[0:1, 1:2]
    a2_ap = consts_sb[0:1, 2:3]
    be2_ap = consts_sb[0:1, 3:4]

    mask_i = p_small.tile([P, TT], mybir.dt.int32)
    nc.sync.dma_start(out=mask_i, in_=mask.rearrange("(t p) -> p t", p=P))
    maskb = p_small.tile([P, TT], f32)
    nc.vector.tensor_copy(out=maskb, in_=mask_i)
    # mask 1 -> 0.0 ; mask 0 -> -1e30  (additive bias inside exp)
    nc.vector.tensor_scalar(
        out=maskb, in0=maskb, scalar1=1e30, scalar2=-1e30, op0=Op.mult, op1=Op.add
    )
    # VectorE fast-exp bias: i16 = s' + (maskb*log2e*128 + EXP_MAGIC) where
    # s' is the pre-scaled score; bitcast of that i16 as bf16 ~ exp(.).
    maskb2 = p_small.tile([P, TT], f32)
    nc.vector.tensor_scalar(
        out=maskb2, in0=maskb, scalar1=float(LOG2E * 128.0), scalar2=EXP_MAGIC,
        op0=Op.mult, op1=Op.add,
    )

    p_xT = tc.alloc_tile_pool(name="p_xT", bufs=1, side="right")
    xT_sb = p_xT.tile([P, KD, S], bf16)
    xT_r = xT.rearrange("(k p) t -> p k t", p=P)
    for t in range(TC):
        sl = slice(t * 512, (t + 1) * 512)
        for k in range(KD):
            nc.sync.dma_start(out=xT_sb[:, k, sl], in_=xT_r[:, k, sl])

    wq_sb = p_wqkv.tile([P, KD, D_MODEL], bf16)
    wk_sb = p_wqkv.tile([P, KD, D_MODEL], bf16)
    wv_sb = p_wqkv.tile([P, KD, D_MODEL], bf16)
    wo_sb = p_wo.tile([P, KD, D_MODEL], bf16)
    for w_sb, w in ((wq_sb, Wq), (wk_sb, Wk), (wv_sb, Wv), (wo_sb, Wo)):
        nc.sync.dma_start(out=w_sb, in_=w.rearrange("(k p) o -> p k o", p=P))
    bq_sb = p_wqkv.tile([P, KD], f32)
    bk_sb = p_wqkv.tile([P, KD], f32)
    bo_sb = p_wo.tile([P, KD], f32)
    nc.sync.dma_start(out=bq_sb, in_=bq.rearrange("(j p) -> p j", p=P))
    nc.sync.dma_start(out=bk_sb, in_=bk.rearrange("(j p) -> p j", p=P))
    nc.sync.dma_start(out=bo_sb, in_=bo.rearrange("(j p) -> p j", p=P))
    bv_f = p_wqkv.tile([1, D_MODEL], f32)
    nc.sync.dma_start(out=bv_f, in_=bv.rearrange("(o d) -> o d", o=1))
    bv_row = p_wqkv.tile([1, D_MODEL], bf16)
    nc.scalar.copy(out=bv_row, in_=bv_f)
    xq_sb = p_xq.tile([P, KD, Q], f32)
    nc.sync.dma_start(out=xq_sb, in_=xq.rearrange("(k p) t -> p k t", p=P))

    # ---------------- LN1 + QKV, pipelined per 512-token chunk ----------
    ps_st = tc.alloc_tile_pool(name="ps_st", bufs=1, space="PSUM")
    ps_b = tc.alloc_tile_pool(name="ps_b", bufs=1, space="PSUM")
    ps_mm = tc.alloc_tile_pool(name="ps_mm", bufs=2, space="PSUM")

    ln1 = p_big.tile([P, KD, S], bf16, tag="big")
    # pool + tiles for W1 sit below p_qkv on the left stack (so p_qkv can be
    # released first); the actual DMA is issued at the attention section.
    p_w1 = tc.alloc_tile_pool(name="p_w1", bufs=1, side="left")
    w1_sb = p_w1.tile([P, KD, D_FF], bf16)
    b1_sb = p_w1.tile([P, FJ], f32)
    p_qkv = tc.alloc_tile_pool(name="p_qkv", bufs=1, side="left")
    qT = p_qkv.tile([P, KD, Q], bf16)
    kT = p_qkv.tile([P, KD, S], bf16)
    v_sb = p_qkv.tile([P, TT, H, DK + 1], bf16)
    nc.gpsimd.memset(v_sb, 1.0)

    LN511 = float(0.5 * np.log(np.float64(D_MODEL - 1)))

    def ln_stats(x_ap, xsq_ap, width, st_pool):
        # stats: column sums of x and x^2 over all KD partition chunks
        s1_ps = st_pool.tile([1, width], f32, tag="s1", bufs=2)
        s2_ps = st_pool.tile([1, width], f32, tag="s2", bufs=2)
        for k in range(KD):
            mm(s1_ps, r(ones_col), r(x_ap[:, k, :]), start=(k == 0),
               stop=(k == KD - 1))
        for k in range(KD):
            mm(s2_ps, r(ones_col), r(xsq_ap[:, k, :]), start=(k == 0),
               stop=(k == KD - 1))
        s1_row = p_work.tile([1, width], f32, tag="s1row", bufs=4)
        s2_row = p_work.tile([1, width], f32, tag="s2row", bufs=4)
        nc.scalar.copy(out=s1_row, in_=s1_ps)
        nc.scalar.copy(out=s2_row, in_=s2_ps)
        return s1_row, s2_row

    def ln_chunk(s1_row, s2_row, width, b_pool, alpha_ap, beta_ap):
        # single-lane f32 coefficient math on [1, width] rows (f32 keeps the
        # s2 - s1^2/n cancellation exact; a row op is only `width` cycles):
        # u = (n-1) var ; 1/std = exp(-.5 ln u + .5 ln(n-1)) ; a = alpha/std
        # c = beta - (s1/n) a
        n_tok = float(D_MODEL)
        t0 = p_work.tile([1, width], f32, tag="lnt0", bufs=2)
        nc.vector.scalar_tensor_tensor(
            out=t0, in0=s1_row, scalar=1.0 / n_tok, in1=s1_row,
            op0=Op.mult, op1=Op.mult,
        )
        u = p_work.tile([1, width], f32, tag="lnu", bufs=2)
        nc.vector.tensor_tensor(out=u, in0=s2_row, in1=t0, op=Op.subtract)
        l = p_work.tile([1, width], f32, tag="lnl", bufs=2)
        nc.scalar.activation(out=l, in_=u, func=Act.Ln)
        a0 = p_work.tile([1, width], f32, tag="lna0", bufs=2)
        nc.scalar.activation(out=a0, in_=l, func=Act.Exp, scale=-0.5, bias=ln511_sb[0:1, 0:1])
        a_row = p_work.tile([1, width], bf16, tag="lnar", bufs=2)
        nc.scalar.mul(out=a_row, in_=a0, mul=alpha_ap)
        cm = p_work.tile([1, width], f32, tag="lncm", bufs=2)
        nc.vector.scalar_tensor_tensor(
            out=cm, in0=s1_row, scalar=-1.0 / n_tok, in1=a_row,
            op0=Op.mult, op1=Op.mult,
        )
        c_row = p_work.tile([1, width], bf16, tag="lncr", bufs=2)
        nc.scalar.add(out=c_row, in_=cm, add=beta_ap)
        # broadcast a/c across partitions (K=1 matmuls), then evict to bf16
        # SBUF (ScalarE) so the wide LN apply runs in the DVE 2x 16-bit mode.
        a_ps = b_pool.tile([P, width], f32, tag="ab")
        c_ps = b_pool.tile([P, width], f32, tag="cb")
        mm(a_ps, r(ones_row), r(a_row), start=True, stop=True)
        mm(c_ps, r(ones_row), r(c_row), start=True, stop=True)
        a_sb = p_work.tile([P, width], bf16, tag="lnasb", bufs=2)
        c_sb = p_work.tile([P, width], bf16, tag="lncsb", bufs=2)
        nc.scalar.copy(out=a_sb, in_=a_ps)
        nc.scalar.copy(out=c_sb, in_=c_ps)
        return a_sb, c_sb

    # all chunks' x^2 + stats first: the PE gets a dense queue of stats
    # matmuls while the first chunks' coefficient chains run on ACT/DVE.
    stat_rows = []
    for t in range(TC):
        sl = slice(t * 512, (t + 1) * 512)
        xsq_t = p_work.tile([P, KD, 512], bf16, tag="xsq", bufs=2)
        for k in range(KD):
            nc.vector.tensor_tensor(
                out=xsq_t[:, k, :], in0=xT_sb[:, k, sl], in1=xT_sb[:, k, sl],
                op=Op.mult,
            )
        stat_rows.append(ln_stats(xT_sb[:, :, sl], xsq_t, 512, ps_st))
    for t in range(TC):
        sl = slice(t * 512, (t + 1) * 512)
        a_sb, c_sb = ln_chunk(
            stat_rows[t][0], stat_rows[t][1], 512, ps_b, a1_ap, be1_ap
        )
        # apply: ln1 = x * a + c  (all-bf16 SBUF operands -> DVE 2x mode)
        for k in range(KD):
            nc.vector.tensor_tensor(
                out=ln1[:, k, sl], in0=xT_sb[:, k, sl], in1=a_sb, op=Op.mult
            )
            nc.vector.tensor_tensor(
                out=ln1[:, k, sl], in0=ln1[:, k, sl], in1=c_sb, op=Op.add
            )
        if t == 0:
            # Q projection only needs the first chunk; qT pre-scaled by EXP_A
            for j in range(KD):
                q_ps = ps_mm.tile([P, 512], f32, tag="mm")
                for k in range(KD):
                    mm(q_ps, r(wq_sb[:, k, j * P:(j + 1) * P]), r(ln1[:, k, 0:Q]),
                       start=(k == 0), stop=(k == KD - 1))
                nc.vector.tensor_scalar(
                    out=qT[:, j, :], in0=q_ps, scalar1=bq_sb[:, j:j + 1],
                    scalar2=EXP_A, op0=Op.add, op1=Op.mult,
                )
        # K projection for this chunk (bias in the DVE eviction)
        for j in range(KD):
            k_ps = ps_mm.tile([P, 512], f32, tag="mm")
            for k in range(KD):
                mm(k_ps, r(wk_sb[:, k, j * P:(j + 1) * P]), r(ln1[:, k, sl]),
                   start=(k == 0), stop=(k == KD - 1))
            nc.vector.tensor_scalar_add(
                out=kT[:, j, sl], in0=k_ps, scalar1=bk_sb[:, j:j + 1]
            )
        # V projection for this chunk (4 kpos tiles of 128 tokens); bv is
        # added by a K=1 ones matmul so the eviction is a plain ScalarE copy
        for tt in range(4 * t, 4 * t + 4):
            v_ps = ps_mm.tile([P, 512], f32, tag="mm")
            for k in range(KD):
                mm(v_ps, r(ln1[:, k, tt * P:(tt + 1) * P]), r(wv_sb[:, k, :]),
                   start=(k == 0), stop=False)
            mm(v_ps, r(ones_row), r(bv_row), start=False, stop=True)
            nc.scalar.copy(
                out=v_sb[:, tt, :, 0:DK],
                in_=v_ps.rearrange("p (h d) -> p h d", h=H),
            )

    p_xT.release()
    p_big.release()
    p_wqkv.release()

    # ---------------- attention ----------------
    ps_mm.release()
    ps_b.release()
    ps_st.release()
    p_attn = tc.alloc_tile_pool(name="p_attn", bufs=1, side="right")
    attn_sb = p_attn.tile([P, KD, Q], bf16)
    ps_sc = tc.alloc_tile_pool(name="ps_sc", bufs=3, space="PSUM")
    ps_ov = tc.alloc_tile_pool(name="ps_ov", bufs=1, space="PSUM")

    # W1 prefetch overlaps attention
    nc.sync.dma_start(out=w1_sb, in_=W1.rearrange("(k p) o -> p k o", p=P))
    nc.sync.dma_start(out=b1_sb, in_=b1.rearrange("(j p) -> p j", p=P))

    # Head pairs: 2 score MMs co-execute on PE row halves into one [P,1024]
    # PSUM tile; exp units alternate between ScalarE (true Exp) and VectorE
    # (bit-trick exp2) so neither engine paces the PE.  The attn@v matmuls
    # are emitted SKEW units behind the scores: the PE queue is in-order, so
    # without the skew a unit's attn@v (waiting on its exp) would block the
    # next unit's already-runnable score matmuls behind it.
    EXP_ACT_UNITS = 33          # of 64 (pair, tt) units on ScalarE
    SKEW = 1
    exp_acc = 0
    ov_tiles = {}
    pending = []

    def emit_attnv(u):
        pj, tt, expT = u
        if tt == 0:
            ov0_t = ps_ov.tile([DK + 1, 512], f32, tag="ov0", name=f"ov0_{pj}")
            ov1_t = ps_ov.tile([DK + 1, 512], f32, tag="ov1", name=f"ov1_{pj}")
            ov_tiles[pj] = (ov0_t, ov1_t)
        ov0, ov1 = ov_tiles[pj]
        mm(ov0, r(v_sb[:, tt, 2 * pj, :]), r(expT[:, 0:512]),
           start=(tt == 0), stop=(tt == TT - 1))
        mm(ov1, r(v_sb[:, tt, 2 * pj + 1, :]), r(expT[:, 512:1024]),
           start=(tt == 0), stop=(tt == TT - 1))
        if tt == TT - 1:
            # epilogue: evict, 1/denominator, broadcast, normalize
            ovs0 = p_attn.tile([DK + 1, 512], f32, tag=f"ovs{pj}_0")
            nc.scalar.copy(out=ovs0, in_=ov0)
            ovs1 = p_attn.tile([DK + 1, 512], f32, tag=f"ovs{pj}_1")
            nc.vector.tensor_copy(out=ovs1, in_=ov1)
            for po, ovs in ((0, ovs0), (DK, ovs1)):
                recip = p_attn.tile([1, 512], f32, tag=f"recip{pj}_{po}")
                nc.scalar.activation(out=recip, in_=ovs[DK:DK + 1, :],
                                     func=Act.Ln)
                nc.scalar.activation(out=recip, in_=recip, func=Act.Exp,
                                     scale=-1.0)
                rb = p_work.tile([DK, 512], f32, tag="rb")
                nc.gpsimd.partition_broadcast(out_ap=rb, in_ap=recip)
                nc.vector.tensor_tensor(
                    out=attn_sb[po:po + DK, pj, :], in0=ovs[0:DK, :], in1=rb,
                    op=Op.mult,
                )

    for pj in range(H // 2):
        for tt in range(TT):
            sc_ps = ps_sc.tile([P, 1024], f32, tag="sc")
            mm(sc_ps[:, 0:512],
               r(kT[0:DK, pj, tt * P:(tt + 1) * P]),
               r(qT[0:DK, pj, :]),
               start=True, stop=True, tile_position=(0, 0))
            mm(sc_ps[:, 512:1024],
               r(kT[DK:P, pj, tt * P:(tt + 1) * P]),
               r(qT[DK:P, pj, :]),
               start=True, stop=True, tile_position=(64, 0))
            # whole [128,1024] exp per unit, alternating engines: large
            # instructions amortize fixed cost vs per-half splits, and
            # keeping combined exp throughput under PE's unit time keeps
            # the PE dense (HAM stays at K=8/8).
            expT = p_work.tile([P, 1024], bf16, tag="expT")
            exp_acc += EXP_ACT_UNITS
            if exp_acc >= 64:
                exp_acc -= 64
                nc.scalar.activation(
                    out=expT, in_=sc_ps, func=Act.Exp,
                    bias=maskb[:, tt:tt + 1], scale=SC_SCALE,
                )
            else:
                nc.vector.tensor_scalar_add(
                    out=expT.bitcast(i16), in0=sc_ps,
                    scalar1=maskb2[:, tt:tt + 1],
                )
            pending.append((pj, tt, expT))
            if len(pending) > SKEW:
                emit_attnv(pending.pop(0))
        # flush at the pair boundary: deferring a pair's last attn@v past the
        # next pair's scores entangles the epilogue with the ov-bank reuse
        # (observed to hang the device at SKEW=2)
        while pending:
            emit_attnv(pending.pop(0))

    ps_ov.release()
    ps_sc.release()
    p_qkv.release()
    ps_mm2 = tc.alloc_tile_pool(name="ps_mm2", bufs=3, space="PSUM")
    ps_st2 = tc.alloc_tile_pool(name="ps_st2", bufs=1, space="PSUM")
    ps_b2 = tc.alloc_tile_pool(name="ps_b2", bufs=1, space="PSUM")

    # W2 load overlaps Wo / LN2
    p_w2 = tc.alloc_tile_pool(name="p_w2", bufs=1, side="left")
    w2_sb = p_w2.tile([P, FJ, D_MODEL], bf16)
    nc.sync.dma_start(out=w2_sb, in_=W2.rearrange("(k p) o -> p k o", p=P))
    b2_sb = p_w2.tile([P, KD], f32)
    nc.sync.dma_start(out=b2_sb, in_=b2.rearrange("(j p) -> p j", p=P))

    # ---------------- Wo + residual -> x2, LN2 stats per j ----------------
    p_x2 = tc.alloc_tile_pool(name="p_x2", bufs=1, side="left")
    x2_sb = p_x2.tile([P, KD, Q], f32)
    x2b = p_x2.tile([P, KD, Q], bf16)
    x2sq = p_x2.tile([P, KD, Q], bf16)
    s1q_ps = ps_st2.tile([1, Q], f32, tag="s1")
    s2q_ps = ps_st2.tile([1, Q], f32, tag="s2")
    for j in range(KD):
        o_ps = ps_mm2.tile([P, 512], f32, tag="mm")
        for k in range(KD):
            mm(o_ps, r(wo_sb[:, k, j * P:(j + 1) * P]), r(attn_sb[:, k, :]),
               start=(k == 0), stop=(k == KD - 1))
        nc.vector.scalar_tensor_tensor(
            out=x2_sb[:, j, :], in0=o_ps, scalar=bo_sb[:, j:j + 1],
            in1=xq_sb[:, j, :], op0=Op.add, op1=Op.add,
        )
        nc.vector.tensor_copy(out=x2b[:, j, :], in_=x2_sb[:, j, :])
        nc.vector.tensor_tensor(
            out=x2sq[:, j, :], in0=x2b[:, j, :], in1=x2b[:, j, :], op=Op.mult
        )
        mm(s1q_ps, r(ones_col), r(x2b[:, j, :]), start=(j == 0),
           stop=(j == KD - 1))
    for j in range(KD):
        mm(s2q_ps, r(ones_col), r(x2sq[:, j, :]), start=(j == 0),
           stop=(j == KD - 1))

    p_attn.release()
    p_xq.release()
    p_wo.release()

    # ---------------- LN2 (same single-lane coeff scheme) ----------------
    s1q_row = p_work.tile([1, Q], f32, tag="s1row", bufs=2)
    s2q_row = p_work.tile([1, Q], f32, tag="s2row", bufs=2)
    nc.scalar.copy(out=s1q_row, in_=s1q_ps)
    nc.scalar.copy(out=s2q_row, in_=s2q_ps)
    n_tok = float(D_MODEL)
    t0 = p_work.tile([1, Q], f32, tag="lnt0", bufs=2)
    nc.vector.scalar_tensor_tensor(
        out=t0, in0=s1q_row, scalar=1.0 / n_tok, in1=s1q_row,
        op0=Op.mult, op1=Op.mult,
    )
    u = p_work.tile([1, Q], f32, tag="lnu", bufs=2)
    nc.vector.tensor_tensor(out=u, in0=s2q_row, in1=t0, op=Op.subtract)
    l = p_work.tile([1, Q], f32, tag="lnl", bufs=2)
    nc.scalar.activation(out=l, in_=u, func=Act.Ln)
    a0 = p_work.tile([1, Q], f32, tag="lna0", bufs=2)
    nc.scalar.activation(out=a0, in_=l, func=Act.Exp, scale=-0.5, bias=ln511_sb[0:1, 0:1])
    a2_row = p_work.tile([1, Q], bf16, tag="lnar", bufs=2)
    nc.scalar.mul(out=a2_row, in_=a0, mul=a2_ap)
    cm = p_work.tile([1, Q], f32, tag="lncm", bufs=2)
    nc.vector.scalar_tensor_tensor(
        out=cm, in0=s1q_row, scalar=-1.0 / n_tok, in1=a2_row,
        op0=Op.mult, op1=Op.mult,
    )
    c2_row = p_work.tile([1, Q], bf16, tag="lncr", bufs=2)
    nc.scalar.add(out=c2_row, in_=cm, add=be2_ap)
    a2_ps = ps_b2.tile([P, Q], f32, tag="ab")
    c2_ps = ps_b2.tile([P, Q], f32, tag="cb")
    mm(a2_ps, r(ones_row), r(a2_row), start=True, stop=True)
    mm(c2_ps, r(ones_row), r(c2_row), start=True, stop=True)
    a2_sb = p_work.tile([P, Q], bf16, tag="lnasb", bufs=2)
    c2_sb = p_work.tile([P, Q], bf16, tag="lncsb", bufs=2)
    nc.scalar.copy(out=a2_sb, in_=a2_ps)
    nc.scalar.copy(out=c2_sb, in_=c2_ps)

    # PE keep-warm: the LN2 coefficient chain leaves the PE idle for several
    # us, long enough for HAM to re-throttle the clock right before the FFN.
    # A few dependency-free dummy matmuls (no reader) keep its activity
    # window busy through the bubble.
    warm_ps = ps_mm2.tile([P, 512], f32, tag="warm", bufs=1)
    for _ in range(8):
        mm(warm_ps[0:1, :], r(ones_col), r(w1_sb[:, 0, 0:512]),
           start=True, stop=True)

    ln2 = p_x2.tile([P, KD, Q], bf16)
    for k in range(KD):
        nc.vector.tensor_tensor(out=ln2[:, k, :], in0=x2b[:, k, :], in1=a2_sb,
                                op=Op.mult)
        nc.vector.tensor_tensor(out=ln2[:, k, :], in0=ln2[:, k, :], in1=c2_sb,
                                op=Op.add)

    # ---------------- FFN ----------------
    p_h = tc.alloc_tile_pool(name="p_h", bufs=1, side="left")
    hT = p_h.tile([P, FJ, Q], bf16)
    for j in range(FJ):
        h_ps = ps_mm2.tile([P, 512], f32, tag="mm")
        for k in range(KD):
            mm(h_ps, r(w1_sb[:, k, j * P:(j + 1) * P]), r(ln2[:, k, :]),
               start=(k == 0), stop=(k == KD - 1))
        nc.scalar.activation(
            out=hT[:, j, :], in_=h_ps, func=Act.Relu, bias=b1_sb[:, j:j + 1],
            scale=1.0,
        )

    for j in range(KD):
        f_ps = ps_mm2.tile([P, 512], f32, tag="mm")
        for k in range(FJ):
            mm(f_ps, r(w2_sb[:, k, j * P:(j + 1) * P]), r(hT[:, k, :]),
               start=(k == 0), stop=(k == FJ - 1))
        o_sb = p_work.tile([P, 512], f32, tag="osb")
        nc.vector.scalar_tensor_tensor(
            out=o_sb, in0=f_ps, scalar=b2_sb[:, j:j + 1], in1=x2_sb[:, j, :],
            op0=Op.add, op1=Op.add,
        )
        nc.sync.dma_start(
            out=outT.rearrange("(j p) q -> p j q", p=P)[:, j, :], in_=o_sb
        )

    for pool in (p_h, p_x2, p_w2, p_w1, ps_b2, ps_st2, p_work, p_small, ps_mm2):
        pool.release()


def _build():
    global _BUILT
    if _BUILT is not None:
        return _BUILT
    import concourse.bass as bass
    import concourse.tile as tile
    from concourse import bacc, mybir
    from concourse._compat import axon_active

    f32 = mybir.dt.float32
    bf16 = mybir.dt.bfloat16
    i32 = mybir.dt.int32
    nc = bacc.Bacc(
        "TRN2",
        target_bir_lowering=False,
        debug=False,
        enable_asserts=False,
        num_devices=N_CORES,
    )
    aps = [
        nc.dram_tensor("xT", [D_MODEL, S], bf16, kind="ExternalInput").ap(),
        nc.dram_tensor("xq", [D_MODEL, Q], f32, kind="ExternalInput").ap(),
        nc.dram_tensor("mask", [S], i32, kind="ExternalInput").ap(),
        nc.dram_tensor("Wq", [D_MODEL, D_MODEL], bf16, kind="ExternalInput").ap(),
        nc.dram_tensor("Wk", [D_MODEL, D_MODEL], bf16, kind="ExternalInput").ap(),
        nc.dram_tensor("Wv", [D_MODEL, D_MODEL], bf16, kind="ExternalInput").ap(),
        nc.dram_tensor("Wo", [D_MODEL, D_MODEL], bf16, kind="ExternalInput").ap(),
        nc.dram_tensor("W1", [D_MODEL, D_FF], bf16, kind="ExternalInput").ap(),
        nc.dram_tensor("W2", [D_FF, D_MODEL], bf16, kind="ExternalInput").ap(),
        nc.dram_tensor("bq", [D_MODEL], f32, kind="ExternalInput").ap(),
        nc.dram_tensor("bk", [D_MODEL], f32, kind="ExternalInput").ap(),
        nc.dram_tensor("bv", [D_MODEL], f32, kind="ExternalInput").ap(),
        nc.dram_tensor("bo", [D_MODEL], f32, kind="ExternalInput").ap(),
        nc.dram_tensor("b1", [D_FF], f32, kind="ExternalInput").ap(),
        nc.dram_tensor("b2", [D_MODEL], f32, kind="ExternalInput").ap(),
        nc.dram_tensor("consts", [4], f32, kind="ExternalInput").ap(),
        nc.dram_tensor("outT", [D_MODEL, Q], f32, kind="ExternalOutput").ap(),
    ]
    with tile.TileContext(nc) as tc:
        _emit(nc, tc, aps)
    # Pin the single ACT table set that contains every function used here
    # (ln, exp, relu, square, identity, copy) so the per-instruction chooser
    # cannot ping-pong between sets (each reload costs ~2.7us serialized on
    # ScalarE).  Indices stay aligned with act_info.json.
    import concourse.bacc as bacc_mod

    orig_tables = bacc_mod.get_activation_tables

    def _one_set_tables(arch):
        tabs = orig_tables(arch)
        keep = "natural_log_exp_and_others"
        return {k: (v if k == keep else set()) for k, v in tabs.items()}

    bacc_mod.get_activation_tables = _one_set_tables
    try:
        nc.compile()
    finally:
        bacc_mod.get_activation_tables = orig_tables
    _BUILT = nc
    return nc


def make_in_maps(inputs):
    import ml_dtypes

    bf16 = ml_dtypes.bfloat16
    x = np.asarray(inputs["x"], np.float32)
    src_mask = np.asarray(inputs["src_mask"], np.int32)
    shared = {
        "Wq": np.ascontiguousarray(np.asarray(inputs["Wq"], np.float32).astype(bf16)),
        "Wk": np.ascontiguousarray(np.asarray(inputs["Wk"], np.float32).astype(bf16)),
        "Wv": np.ascontiguousarray(np.asarray(inputs["Wv"], np.float32).astype(bf16)),
        "Wo": np.ascontiguousarray(np.asarray(inputs["Wo"], np.float32).astype(bf16)),
        "W1": np.ascontiguousarray(np.asarray(inputs["W1"], np.float32).astype(bf16)),
        "W2": np.ascontiguousarray(np.asarray(inputs["W2"], np.float32).astype(bf16)),
        "bq": np.ascontiguousarray(np.asarray(inputs["bq"], np.float32)),
        "bk": np.ascontiguousarray(np.asarray(inputs["bk"], np.float32)),
        "bv": np.ascontiguousarray(np.asarray(inputs["bv"], np.float32)),
        "bo": np.ascontiguousarray(np.asarray(inputs["bo"], np.float32)),
        "b1": np.ascontiguousarray(np.asarray(inputs["b1"], np.float32)),
        "b2": np.ascontiguousarray(np.asarray(inputs["b2"], np.float32)),
        "consts": np.ascontiguousarray(
            np.array(
                [
                    np.asarray(inputs["alpha1"]).reshape(-1)[0],
                    np.asarray(inputs["beta1"]).reshape(-1)[0],
                    np.asarray(inputs["alpha2"]).reshape(-1)[0],
                    np.asarray(inputs["beta2"]).reshape(-1)[0],
                ],
                np.float32,
            )
        ),
    }
    in_maps = []
    for c in range(N_CORES):
        b = c // CORES_PER_BATCH
        qs = (c % CORES_PER_BATCH) * Q
        x_rot = np.concatenate([x[b, qs:, :], x[b, :qs, :]], axis=0)
        m_b = src_mask[b, 0, 0, :]
        m_rot = np.concatenate([m_b[qs:], m_b[:qs]], axis=0)
        in_map = dict(shared)
        in_map["xT"] = np.ascontiguousarray(x_rot.T.astype(bf16))
        in_map["xq"] = np.ascontiguousarray(x_rot[0:Q, :].T)
        in_map["mask"] = np.ascontiguousarray(m_rot)
        in_maps.append(in_map)
    return in_maps


def assemble_output(results):
    out = np.empty((B, S, D_MODEL), np.float32)
    for c in range(N_CORES):
        b = c // CORES_PER_BATCH
        qs = (c % CORES_PER_BATCH) * Q
        out[b, qs:qs + Q, :] = results[c]["outT"].T
    return out


def kernel(**inputs):
    from concourse.bass_utils import run_bass_kernel_spmd

    nc = _build()
    in_maps = make_in_maps(inputs)
    res = run_bass_kernel_spmd(nc, in_maps, core_ids=list(range(N_CORES)))
    return assemble_output(res.results)
